# revision 37
# baseline (speedup 1.0000x reference)
"""Trainium2 Bass kernel for the Cross_Attention module.

Math (per batch b, per output stream):
  f1 = Wf1 @ x + bf1         [D, N]   (from x, both streams)
  f2 = Wf2 @ y + bf2         [D, N]   (from y, both streams)
  g  = Wg  @ z + bg          [D, N]   (z = x for the x_out stream, y for y_out)
  h  = Wh  @ x + bh          [D, N]   (always from x)
  A_a[i, j] = softmax_j(f_a[:, i] . g[:, j])        a in {1, 2}
  out = z + ga * (Wv1 @ (h A_1^T) + bv1) + gb * (Wv2 @ (h A_2^T) + bv2)

Sharding: 8 cores = 4 batches x 2 streams (x_out / y_out). No collectives.

Device algorithm (per core):
  - logits computed TRANSPOSED: LT[j, i] = sum_d g[d, j] f[d, i] so that the
    softmax reduction axis j lands on PSUM partitions. The two attentions
    share the stationary g tile and run as row-tiled bf16 matmuls
    (f1 in partitions 0:64, f2 in 64:128).
  - E = exp(LT - 40)  (constant shift; |logits| << 40+88 so exp is safe, and
    softmax is shift-invariant so the result is exact).
  - num[., i] = [hT | ones]^T @ E: one matmul per j-tile accumulates both the
    numerator (rows 0..63) and the softmax denominator (row 64).
  - EA = num[0:64] * (1/num[64]) broadcast via a K=2 selector matmul that
    serves both attentions at once.
  - out = z + [ga*Wv1 | gb*Wv2] @ [EA1; EA2] + (ga*bv1 + gb*bv2): the two
    value GEMMs are one K=128 matmul against host-stacked weights.
Projection chains are merged ([Wf1|Wh] is one K=128-wide stationary) and
pipelined in 256-column chunks against the input DMA stream, so compute
starts ~3us in and the attention loop is paced by the Activation engine
(the exp of 2*N^2 logits is the hard floor of this problem).
"""

import numpy as np

import concourse.bass as bass
import concourse.bacc as bacc
import concourse.mybir as mybir
import concourse.tile as tile
from concourse.masks import make_identity

BS = 4
C = 512
D = 64
H = W = 48
N = H * W          # 2304
P = 128
NK = C // P        # 4 contraction tiles for the projections
NCT = C // P       # 4 output channel tiles
NJT = N // P       # 18 j tiles
IBLK = 512
IBLOCKS = [(0, 512), (512, 512), (1024, 512), (1536, 512), (2048, 256)]
CHUNK = 256        # projection / input streaming chunk (columns)
NCH = N // CHUNK   # 9
SHIFT = 40.0

F32 = mybir.dt.float32
F32R = mybir.dt.float32r
BF16 = mybir.dt.bfloat16
AF = mybir.ActivationFunctionType
OP = mybir.AluOpType


def build_program():
    nc = bacc.Bacc("TRN2", target_bir_lowering=False)

    xin = nc.dram_tensor("xin", [C, N], BF16, kind="ExternalInput")
    yin = nc.dram_tensor("yin", [C, N], BF16, kind="ExternalInput")
    zin = nc.dram_tensor("zin", [C, N], BF16, kind="ExternalInput")
    # host-marshalled weights: Wf1h = [Wf1.T | Wh.T] (f1+h share one chain),
    # WvS = [ga*Wv1.T ; gb*Wv2.T] stacked on the contraction dim,
    # smalls = biases + cv packed: col0=[bf1;bh] col1=[bf2;-] col2=[bg;-]
    # cols 4:8 = cv = ga*bv1 + gb*bv2 in (ci, ct) layout.
    # Wcat = [WgT | Wf1T | WhT | Wf2T] in one DMA-friendly block
    Wcat = nc.dram_tensor("Wcat", [C, 4 * D], BF16, kind="ExternalInput")
    WvS = nc.dram_tensor("WvS", [P, C], F32, kind="ExternalInput")
    smalls = nc.dram_tensor("smalls", [P, 8], F32, kind="ExternalInput")
    sel = nc.dram_tensor("sel", [2, P], F32, kind="ExternalInput")
    out = nc.dram_tensor("out", [C, N], F32, kind="ExternalOutput")

    xin_r = xin.rearrange("(co ci) n -> ci co n", ci=P)
    yin_r = yin.rearrange("(co ci) n -> ci co n", ci=P)
    zin_r = zin.rearrange("(co ci) n -> ci co n", ci=P)
    out_r = out.rearrange("(co ci) n -> ci co n", ci=P)

    with tile.TileContext(nc) as tc:
        with (
            tc.tile_pool(name="persist", bufs=1) as persist,
            tc.tile_pool(name="scratch", bufs=2, space="PSUM") as scratch,
            tc.tile_pool(name="ltp", bufs=2, space="PSUM") as ltp,
            tc.tile_pool(name="nump", bufs=2, space="PSUM") as nump,
            tc.tile_pool(name="ebuf", bufs=3) as ebuf,
            tc.tile_pool(name="eap", bufs=2) as eap,
            tc.tile_pool(name="rcpp", bufs=2) as rcpp,
            tc.tile_pool(name="osbp", bufs=4) as osbp,
        ):
            xin_sb = persist.tile([P, NK, N], BF16)
            yin_sb = persist.tile([P, NK, N], BF16)
            zin_sb = persist.tile([P, NK, N], BF16)

            # projection weights + biases go FIRST on the sync queue: small
            # (0.45 MB) but they gate the first projection chains; the input
            # chunks stream right behind them
            # weights issue from the Act queue so the first z/x/y chunks own
            # the sync queue (and thus the head of the DMA-engine line)
            smalls_sb = persist.tile([P, 8], F32)
            nc.scalar.dma_start(out=smalls_sb, in_=smalls[:, :])
            Wcat_sb = persist.tile([P, NK, 4 * D], BF16)
            nc.scalar.dma_start(
                out=Wcat_sb, in_=Wcat.rearrange("(k ci) d -> ci k d", ci=P))
            WgT_sb = Wcat_sb[:, :, 0:D]
            Wf1h_sb = Wcat_sb[:, :, D:3 * D]
            Wf2T_sb = Wcat_sb[:, :, 3 * D:4 * D]
            # value weights + selector are needed only ~25us in; they ride
            # the gpsimd (SWDGE) queue
            WvS_sb = persist.tile([P, NCT, P], F32R)
            nc.gpsimd.dma_start(
                out=WvS_sb,
                in_=WvS.rearrange("d (ct ci) -> d ct ci", ci=P).bitcast(F32R))

            # inputs stream in CHUNK-col slices; z first (g chain gates the
            # logits), then x (f1+h), then y (f2)
            for ch in range(NCH):
                sl = slice(ch * CHUNK, (ch + 1) * CHUNK)
                nc.sync.dma_start(out=zin_sb[:, :, sl], in_=zin_r[:, :, sl])
                nc.sync.dma_start(out=xin_sb[:, :, sl], in_=xin_r[:, :, sl])
                nc.sync.dma_start(out=yin_sb[:, :, sl], in_=yin_r[:, :, sl])

            # ---------------- constants ----------------
            # identity placed at partitions 64:128 (transposes read h from
            # the upper half of the f1h tile): ident2[x, y] = 1 iff x-64 == y
            ident2 = persist.tile([P, D], BF16)
            nc.gpsimd.memset(ident2, 0.0)
            nc.gpsimd.affine_select(
                out=ident2, in_=ident2,
                compare_op=mybir.AluOpType.not_equal, fill=1.0,
                base=-D, pattern=[[-1, D]], channel_multiplier=1)
            onesF = persist.tile([P, 1], F32)
            nc.vector.memset(onesF, 1.0)
            shiftb = persist.tile([P, 1], F32)
            nc.vector.memset(shiftb, -SHIFT)
            # dummy 1-element exp: pulls the ACT table load off the critical
            # path (runs during the input DMA head)
            dummy = persist.tile([1, 1], F32)
            nc.scalar.activation(
                out=dummy[0:1, 0:1], in_=shiftb[0:1, 0:1], func=AF.Exp,
                bias=shiftb[0:1, 0:1], scale=1.0)
            # selector for the K=2 reciprocal broadcast: row0 -> parts 0:64,
            # row1 -> parts 64:128 (host-supplied 0/1 matrix)
            sel2 = persist.tile([2, P], F32R)
            nc.gpsimd.dma_start(out=sel2, in_=sel[:, :].bitcast(F32R))

            # ---------------- persistent activations ----------------
            f1h_sb = persist.tile([P, N], BF16)   # rows 0:64 f1, 64:128 h
            f2_sb = persist.tile([D, N], BF16)
            g_sb = persist.tile([D, N], BF16)
            hT_sb = persist.tile([P, NJT, D + 1], BF16)
            nc.vector.tensor_copy(
                out=hT_sb[:, :, D],
                in_=onesF[:, 0:1].broadcast_to([P, NJT]))

            def emit_proj(ch):
                """Projection chains + hT transposes for a 256-col chunk."""
                i0 = ch * CHUNK
                w = CHUNK
                sl = slice(i0, i0 + w)
                pg = scratch.tile([P, IBLK], F32, tag="sc", name="pg")
                for k in range(NK):
                    nc.tensor.matmul(
                        pg[0:D, :w], lhsT=WgT_sb[:, k, :],
                        rhs=zin_sb[:, k, sl], start=(k == 0), stop=(k == NK - 1))
                nc.vector.tensor_scalar_add(
                    out=g_sb[:, sl], in0=pg[0:D, :w],
                    scalar1=smalls_sb[0:D, 2:3])
                pf = scratch.tile([P, IBLK], F32, tag="sc", name="pf")
                for k in range(NK):
                    nc.tensor.matmul(
                        pf[:, :w], lhsT=Wf1h_sb[:, k, :],
                        rhs=xin_sb[:, k, sl], start=(k == 0), stop=(k == NK - 1))
                nc.vector.tensor_scalar_add(
                    out=f1h_sb[:, sl], in0=pf[:, :w],
                    scalar1=smalls_sb[:, 0:1])
                pq = scratch.tile([P, IBLK], F32, tag="sc", name="pq")
                for k in range(NK):
                    nc.tensor.matmul(
                        pq[0:D, :w], lhsT=Wf2T_sb[:, k, :],
                        rhs=yin_sb[:, k, sl], start=(k == 0), stop=(k == NK - 1))
                nc.vector.tensor_scalar_add(
                    out=f2_sb[:, sl], in0=pq[0:D, :w],
                    scalar1=smalls_sb[0:D, 1:2])

            def emit_logits_half(lt, et, jt, h0, hw):
                j0 = jt * P
                hsl = slice(h0, h0 + hw)
                nc.tensor.matmul(
                    lt[:, 0, hsl], lhsT=g_sb[:, j0:j0 + P],
                    rhs=f1h_sb[0:D, hsl], start=True, stop=True)
                nc.tensor.matmul(
                    lt[:, 1, hsl], lhsT=g_sb[:, j0:j0 + P],
                    rhs=f2_sb[:, hsl], start=True, stop=True)
                nc.scalar.activation(
                    out=et[:, :, hsl], in_=lt[:, :, hsl],
                    func=AF.Exp, bias=shiftb[:, 0:1], scale=1.0)

            def emit_transposes(ch):
                i0 = ch * CHUNK
                for jt in range(i0 // P, (i0 + CHUNK) // P):
                    pT = scratch.tile([P, IBLK], BF16, tag="sc", name="pT")
                    nc.tensor.transpose(
                        pT[:, 0:D], f1h_sb[D:P, jt * P:(jt + 1) * P],
                        ident2[D:P, 0:D])
                    nc.vector.tensor_copy(
                        out=hT_sb[:, jt, 0:D], in_=pT[:, 0:D])

            emit_proj(0)

            # ---------------- output projection ----------------
            # one channel-tile per call so the work spreads across j-loop
            # iterations instead of stalling PE in one burst
            def emit_out_start(ea, i0, w):
                return {"ea": ea, "i0": i0, "w": w}

            def emit_out_ct(st, ct):
                ea, i0, w = st["ea"], st["i0"], st["w"]
                osb = osbp.tile([P, IBLK], F32, tag="osb", name="osb")
                # osb = z + cv first (waits only on the input stream),
                # then += the value GEMM result
                nc.vector.tensor_scalar_add(
                    out=osb[:, :w], in0=zin_sb[:, ct, i0:i0 + w],
                    scalar1=smalls_sb[:, 4 + ct:5 + ct])
                ops = scratch.tile([P, IBLK], F32, tag="sc", name="ops")
                nc.tensor.matmul(
                    ops[:, :w], lhsT=WvS_sb[:, ct, :],
                    rhs=ea[:, :w], start=True, stop=True)
                nc.vector.tensor_add(
                    osb[:, :w], osb[:, :w], ops[:, :w])
                nc.sync.dma_start(
                    out=out_r[:, ct, i0:i0 + w], in_=osb[:, :w])

            # ---------------- attention main loop ----------------
            pending = None
            for ib, (i0, w) in enumerate(IBLOCKS):
                isl = slice(i0, i0 + w)
                num1 = nump.tile([D + 1, IBLK], F32, tag="num", name="num1")
                num2 = nump.tile([D + 1, IBLK], F32, tag="num", name="num2")
                prev_et = None
                # num consumes et with a 2-iteration lag so the logits for
                # jt+1 are never queued behind matmuls that wait on a fresh
                # exp result (PE stays an exp ahead of Act)
                et_hist = [None, None]
                jt_start = 0
                if ib == 0:
                    # prologue: first two j-tiles in 256-col halves, pipelined
                    # against the arrival of projection chunks 1 and 2 so the
                    # first exps fire as soon as chunk 0 is projected
                    lt0 = ltp.tile([P, 2, IBLK], F32, tag="lt", name="lt")
                    et0 = ebuf.tile([P, 2, IBLK], BF16, tag="et", name="et")
                    lt1 = ltp.tile([P, 2, IBLK], F32, tag="lt", name="lt")
                    et1 = ebuf.tile([P, 2, IBLK], BF16, tag="et", name="et")
                    emit_logits_half(lt0, et0, 0, 0, CHUNK)
                    emit_logits_half(lt1, et1, 1, 0, CHUNK)
                    emit_proj(1)
                    emit_transposes(0)
                    emit_transposes(1)
                    emit_logits_half(lt0, et0, 0, CHUNK, CHUNK)
                    emit_logits_half(lt1, et1, 1, CHUNK, CHUNK)
                    emit_proj(2)
                    emit_transposes(2)
                    et_hist = [et0, et1]
                    jt_start = 2
                for jt in range(jt_start, NJT + 2):
                    if ib == 0 and jt % 2 == 0 and 4 <= jt <= 14:
                        emit_proj(jt // 2 + 1)
                        emit_transposes(jt // 2 + 1)
                    if jt < NJT:
                        j0 = jt * P
                        lt = ltp.tile([P, 2, IBLK], F32, tag="lt", name="lt")
                        et = ebuf.tile([P, 2, IBLK], BF16, tag="et", name="et")
                        nc.tensor.matmul(
                            lt[:, 0, :w], lhsT=g_sb[:, j0:j0 + P],
                            rhs=f1h_sb[0:D, isl], start=True, stop=True)
                        nc.tensor.matmul(
                            lt[:, 1, :w], lhsT=g_sb[:, j0:j0 + P],
                            rhs=f2_sb[:, isl], start=True, stop=True)
                        nc.scalar.activation(
                            out=et[:, :, :w], in_=lt[:, :, :w],
                            func=AF.Exp, bias=shiftb[:, 0:1], scale=1.0)
                    if jt >= 2:
                        pj = jt - 2
                        st, sp = (pj == 0), (pj == NJT - 1)
                        nc.tensor.matmul(
                            num1[:, :w], lhsT=hT_sb[:, pj, :],
                            rhs=et_hist[0][:, 0, :w], start=st, stop=sp)
                        nc.tensor.matmul(
                            num2[:, :w], lhsT=hT_sb[:, pj, :],
                            rhs=et_hist[0][:, 1, :w], start=st, stop=sp)
                    if jt < NJT:
                        et_hist = [et_hist[1], et]
                    else:
                        et_hist = [et_hist[1], None]
                    if pending is not None and jt in (6, 8, 10, 12):
                        emit_out_ct(pending, (jt - 6) // 2)
                        if jt == 12:
                            pending = None
                rcp1 = rcpp.tile([1, IBLK], F32R, tag="rcp", name="rcp1")
                rcp2 = rcpp.tile([1, IBLK], F32R, tag="rcp", name="rcp2")
                with nc.allow_low_precision(
                        reason="softmax denominator reciprocal in f32r"):
                    nc.vector.reciprocal(rcp1[0:1, :w], num1[D:D + 1, :w])
                    nc.vector.reciprocal(rcp2[0:1, :w], num2[D:D + 1, :w])
                rb1 = scratch.tile([P, IBLK], F32, tag="sc", name="rb1")
                nc.tensor.matmul(
                    rb1[0:D, :w], lhsT=sel2[0:1, 0:D], rhs=rcp1[:, :w],
                    start=True, stop=True)
                rb2 = scratch.tile([P, IBLK], F32, tag="sc", name="rb2")
                nc.tensor.matmul(
                    rb2[0:D, :w], lhsT=sel2[0:1, 0:D], rhs=rcp2[:, :w],
                    start=True, stop=True)
                ea = eap.tile([P, IBLK], F32R, tag="ea", name="ea")
                if ib == len(IBLOCKS) - 1:
                    # last block: Act is idle after the final exp — run the
                    # copies there so DVE only serializes rcp+mul in the tail
                    nc.scalar.activation(
                        out=ea[0:D, :w], in_=num1[0:D, :w], func=AF.Copy)
                    nc.scalar.activation(
                        out=ea[D:P, :w], in_=num2[0:D, :w], func=AF.Copy)
                else:
                    nc.vector.tensor_copy(out=ea[0:D, :w], in_=num1[0:D, :w])
                    nc.vector.tensor_copy(out=ea[D:P, :w], in_=num2[0:D, :w])
                nc.vector.tensor_mul(ea[0:D, :w], ea[0:D, :w], rb1[0:D, :w])
                nc.vector.tensor_mul(ea[D:P, :w], ea[D:P, :w], rb2[0:D, :w])
                pending = emit_out_start(ea, i0, w)
            for ct in range(NCT):
                emit_out_ct(pending, ct)

    nc.compile()
    return nc


_NC_CACHE = None


def _get_nc():
    global _NC_CACHE
    if _NC_CACHE is None:
        _NC_CACHE = build_program()
    return _NC_CACHE


def _run(inputs, trace=False, trace_cores=None):
    from concourse.bass_utils import run_bass_kernel_spmd

    import ml_dtypes
    g = {k: np.ascontiguousarray(np.asarray(v, dtype=np.float32))
         for k, v in inputs.items()}
    x = g["x"].reshape(BS, C, N)
    y = g["y"].reshape(BS, C, N)
    x16 = np.ascontiguousarray(x.astype(ml_dtypes.bfloat16))
    y16 = np.ascontiguousarray(y.astype(ml_dtypes.bfloat16))

    def core_inputs(b, s):
        def sel(a0, a1):
            return a0 if s == 0 else a1

        gate1 = float(np.asarray(sel(g["alpha"], g["gamma"])).reshape(-1)[0])
        gate2 = float(np.asarray(sel(g["beta"], g["sigma"])).reshape(-1)[0])
        Wcat = np.concatenate(
            [sel(g["Wg1"], g["Wg2"]).T, g["Wf1"].T,
             sel(g["Wh1"], g["Wh2"]).T, g["Wf2"].T], axis=1)   # [C, 256]
        WvS = np.concatenate(
            [gate1 * sel(g["Wv11"], g["Wv12"]).T,
             gate2 * sel(g["Wv21"], g["Wv22"]).T], axis=0)     # [128, C]
        cv = (gate1 * sel(g["bv11"], g["bv12"])
              + gate2 * sel(g["bv21"], g["bv22"]))             # [C]
        smalls = np.zeros((P, 8), np.float32)
        smalls[0:D, 0] = g["bf1"]
        smalls[D:P, 0] = sel(g["bh1"], g["bh2"])
        smalls[0:D, 1] = g["bf2"]
        smalls[0:D, 2] = sel(g["bg1"], g["bg2"])
        smalls[:, 4:8] = cv.reshape(NCT, P).T
        sel2 = np.zeros((2, P), np.float32)
        sel2[0, 0:D] = 1.0
        sel2[1, D:P] = 1.0
        return {
            "xin": np.ascontiguousarray(x16[b]),
            "yin": np.ascontiguousarray(y16[b]),
            "zin": np.ascontiguousarray(sel(x16, y16)[b]),
            "Wcat": np.ascontiguousarray(Wcat.astype(ml_dtypes.bfloat16)),
            "WvS": np.ascontiguousarray(WvS),
            "smalls": smalls,
            "sel": sel2,
        }

    in_maps = [core_inputs(core // 2, core % 2) for core in range(8)]
    res = run_bass_kernel_spmd(
        _get_nc(), in_maps, core_ids=list(range(8)), trace=trace,
        trace_cores=trace_cores)
    outs = [r["out"] for r in res.results]
    x_out = np.stack([outs[2 * b] for b in range(BS)]).reshape(BS, C, H, W)
    y_out = np.stack([outs[2 * b + 1] for b in range(BS)]).reshape(BS, C, H, W)
    return (x_out, y_out), res


def kernel(**inputs):
    out, _ = _run(inputs)
    return out


# revision 41
# speedup vs baseline: 1.0167x; 1.0167x over previous
"""Trainium2 Bass kernel for the Cross_Attention module.

Math (per batch b, per output stream):
  f1 = Wf1 @ x + bf1         [D, N]   (from x, both streams)
  f2 = Wf2 @ y + bf2         [D, N]   (from y, both streams)
  g  = Wg  @ z + bg          [D, N]   (z = x for the x_out stream, y for y_out)
  h  = Wh  @ x + bh          [D, N]   (always from x)
  A_a[i, j] = softmax_j(f_a[:, i] . g[:, j])        a in {1, 2}
  out = z + ga * (Wv1 @ (h A_1^T) + bv1) + gb * (Wv2 @ (h A_2^T) + bv2)

Sharding: 8 cores = 4 batches x 2 streams (x_out / y_out). No collectives.

Device algorithm (per core):
  - logits computed TRANSPOSED: LT[j, i] = sum_d g[d, j] f[d, i] so that the
    softmax reduction axis j lands on PSUM partitions. The two attentions
    share the stationary g tile and run as row-tiled bf16 matmuls
    (f1 in partitions 0:64, f2 in 64:128).
  - E = exp(LT - 40)  (constant shift; |logits| << 40+88 so exp is safe, and
    softmax is shift-invariant so the result is exact).
  - num[., i] = [hT | ones]^T @ E: one matmul per j-tile accumulates both the
    numerator (rows 0..63) and the softmax denominator (row 64).
  - EA = num[0:64] * (1/num[64]) broadcast via a K=2 selector matmul that
    serves both attentions at once.
  - out = z + [ga*Wv1 | gb*Wv2] @ [EA1; EA2] + (ga*bv1 + gb*bv2): the two
    value GEMMs are one K=128 matmul against host-stacked weights.
Projection chains are merged ([Wf1|Wh] is one K=128-wide stationary) and
pipelined in 256-column chunks against the input DMA stream, so compute
starts ~3us in and the attention loop is paced by the Activation engine
(the exp of 2*N^2 logits is the hard floor of this problem).
"""

import numpy as np

import concourse.bass as bass
import concourse.bacc as bacc
import concourse.mybir as mybir
import concourse.tile as tile
from concourse.masks import make_identity

BS = 4
C = 512
D = 64
H = W = 48
N = H * W          # 2304
P = 128
NK = C // P        # 4 contraction tiles for the projections
NCT = C // P       # 4 output channel tiles
NJT = N // P       # 18 j tiles
IBLK = 512
IBLOCKS = [(0, 512), (512, 512), (1024, 512), (1536, 512), (2048, 256)]
CHUNK = 256        # projection / input streaming chunk (columns)
NCH = N // CHUNK   # 9
SHIFT = 40.0

F32 = mybir.dt.float32
F32R = mybir.dt.float32r
BF16 = mybir.dt.bfloat16
AF = mybir.ActivationFunctionType
OP = mybir.AluOpType


def build_program():
    nc = bacc.Bacc("TRN2", target_bir_lowering=False)

    xin = nc.dram_tensor("xin", [C, N], BF16, kind="ExternalInput")
    yin = nc.dram_tensor("yin", [C, N], BF16, kind="ExternalInput")
    zin = nc.dram_tensor("zin", [C, N], BF16, kind="ExternalInput")
    # host-marshalled weights: Wf1h = [Wf1.T | Wh.T] (f1+h share one chain),
    # WvS = [ga*Wv1.T ; gb*Wv2.T] stacked on the contraction dim,
    # smalls = biases + cv packed: col0=[bf1;bh] col1=[bf2;-] col2=[bg;-]
    # cols 4:8 = cv = ga*bv1 + gb*bv2 in (ci, ct) layout.
    # Wcat = [WgT | Wf1T | WhT | Wf2T] in one DMA-friendly block
    Wcat = nc.dram_tensor("Wcat", [C, 4 * D], BF16, kind="ExternalInput")
    WvS = nc.dram_tensor("WvS", [P, C], F32, kind="ExternalInput")
    smalls = nc.dram_tensor("smalls", [P, 8], F32, kind="ExternalInput")
    sel = nc.dram_tensor("sel", [2, P], F32, kind="ExternalInput")
    out = nc.dram_tensor("out", [C, N], F32, kind="ExternalOutput")

    xin_r = xin.rearrange("(co ci) n -> ci co n", ci=P)
    yin_r = yin.rearrange("(co ci) n -> ci co n", ci=P)
    zin_r = zin.rearrange("(co ci) n -> ci co n", ci=P)
    out_r = out.rearrange("(co ci) n -> ci co n", ci=P)

    with tile.TileContext(nc) as tc:
        with (
            tc.tile_pool(name="persist", bufs=1) as persist,
            tc.tile_pool(name="scratch", bufs=2, space="PSUM") as scratch,
            tc.tile_pool(name="ltp", bufs=2, space="PSUM") as ltp,
            tc.tile_pool(name="nump", bufs=2, space="PSUM") as nump,
            tc.tile_pool(name="ebuf", bufs=3) as ebuf,
            tc.tile_pool(name="eap", bufs=2) as eap,
            tc.tile_pool(name="rcpp", bufs=2) as rcpp,
            tc.tile_pool(name="osbp", bufs=4) as osbp,
        ):
            xin_sb = persist.tile([P, NK, N], BF16)
            yin_sb = persist.tile([P, NK, N], BF16)
            zin_sb = persist.tile([P, NK, N], BF16)

            # identity at partitions 64:128 (transposes read h from the upper
            # half of the f1h tile): ident2[x, y] = 1 iff x-64 == y
            ident2 = persist.tile([P, D], BF16)
            nc.gpsimd.memset(ident2, 0.0)
            nc.gpsimd.affine_select(
                out=ident2, in_=ident2,
                compare_op=mybir.AluOpType.not_equal, fill=1.0,
                base=-D, pattern=[[-1, D]], channel_multiplier=1)
            # PE p-state warmup: the tensor engine clock ramps only under
            # continuous execution; a dozen throwaway matmuls during the DMA
            # head mean the first projection chains run at full speed
            warmW = persist.tile([P, IBLK], BF16)
            nc.vector.memset(warmW, 0.0)
            for _ in range(12):
                pW = scratch.tile([P, IBLK], F32, tag="sc", name="pW")
                nc.tensor.matmul(
                    pW[0:D, :], lhsT=ident2[D:P, 0:D], rhs=warmW[D:P, :],
                    start=True, stop=True)

            # weights issue from the Act queue so the first z/x/y chunks own
            # the sync queue (and thus the head of the DMA-engine line)
            Wcat_sb = persist.tile([P, NK, 4 * D], BF16)
            nc.scalar.dma_start(
                out=Wcat_sb, in_=Wcat.rearrange("(k ci) d -> ci k d", ci=P))
            smalls_sb = persist.tile([P, 8], F32)
            nc.scalar.dma_start(out=smalls_sb, in_=smalls[:, :])
            WgT_sb = Wcat_sb[:, :, 0:D]
            Wf1h_sb = Wcat_sb[:, :, D:3 * D]
            Wf2T_sb = Wcat_sb[:, :, 3 * D:4 * D]
            # value weights + selector are needed only ~25us in; they ride
            # the gpsimd (SWDGE) queue
            WvS_sb = persist.tile([P, NCT, P], F32R)
            nc.gpsimd.dma_start(
                out=WvS_sb,
                in_=WvS.rearrange("d (ct ci) -> d ct ci", ci=P).bitcast(F32R))

            # inputs stream in CHUNK-col slices; z first (g chain gates the
            # logits), then x (f1+h), then y (f2)
            for ch in range(NCH):
                sl = slice(ch * CHUNK, (ch + 1) * CHUNK)
                nc.sync.dma_start(out=zin_sb[:, :, sl], in_=zin_r[:, :, sl])
                nc.sync.dma_start(out=xin_sb[:, :, sl], in_=xin_r[:, :, sl])
                nc.sync.dma_start(out=yin_sb[:, :, sl], in_=yin_r[:, :, sl])

            # ---------------- constants ----------------
            onesF = persist.tile([P, 1], F32)
            nc.vector.memset(onesF, 1.0)
            shiftb = persist.tile([P, 1], F32)
            nc.vector.memset(shiftb, -SHIFT)
            # dummy 1-element exp: pulls the ACT table load off the critical
            # path (runs during the input DMA head)
            dummy = persist.tile([1, 1], F32)
            nc.scalar.activation(
                out=dummy[0:1, 0:1], in_=shiftb[0:1, 0:1], func=AF.Exp,
                bias=shiftb[0:1, 0:1], scale=1.0)
            # selector for the K=2 reciprocal broadcast: row0 -> parts 0:64,
            # row1 -> parts 64:128 (host-supplied 0/1 matrix)
            sel2 = persist.tile([2, P], F32R)
            nc.gpsimd.dma_start(out=sel2, in_=sel[:, :].bitcast(F32R))

            # ---------------- persistent activations ----------------
            f1h_sb = persist.tile([P, N], BF16)   # rows 0:64 f1, 64:128 h
            f2_sb = persist.tile([D, N], BF16)
            g_sb = persist.tile([D, N], BF16)
            hT_sb = persist.tile([P, NJT, D + 1], BF16)
            nc.vector.tensor_copy(
                out=hT_sb[:, :, D],
                in_=onesF[:, 0:1].broadcast_to([P, NJT]))

            def emit_proj(ch):
                """Projection chains + hT transposes for a 256-col chunk."""
                i0 = ch * CHUNK
                w = CHUNK
                sl = slice(i0, i0 + w)
                pg = scratch.tile([P, IBLK], F32, tag="sc", name="pg")
                for k in range(NK):
                    nc.tensor.matmul(
                        pg[0:D, :w], lhsT=WgT_sb[:, k, :],
                        rhs=zin_sb[:, k, sl], start=(k == 0), stop=(k == NK - 1))
                nc.vector.tensor_scalar_add(
                    out=g_sb[:, sl], in0=pg[0:D, :w],
                    scalar1=smalls_sb[0:D, 2:3])
                pf = scratch.tile([P, IBLK], F32, tag="sc", name="pf")
                for k in range(NK):
                    nc.tensor.matmul(
                        pf[:, :w], lhsT=Wf1h_sb[:, k, :],
                        rhs=xin_sb[:, k, sl], start=(k == 0), stop=(k == NK - 1))
                nc.vector.tensor_scalar_add(
                    out=f1h_sb[:, sl], in0=pf[:, :w],
                    scalar1=smalls_sb[:, 0:1])
                pq = scratch.tile([P, IBLK], F32, tag="sc", name="pq")
                for k in range(NK):
                    nc.tensor.matmul(
                        pq[0:D, :w], lhsT=Wf2T_sb[:, k, :],
                        rhs=yin_sb[:, k, sl], start=(k == 0), stop=(k == NK - 1))
                nc.vector.tensor_scalar_add(
                    out=f2_sb[:, sl], in0=pq[0:D, :w],
                    scalar1=smalls_sb[0:D, 1:2])

            def emit_logits_half(lt, et, jt, h0, hw):
                j0 = jt * P
                hsl = slice(h0, h0 + hw)
                nc.tensor.matmul(
                    lt[:, 0, hsl], lhsT=g_sb[:, j0:j0 + P],
                    rhs=f1h_sb[0:D, hsl], start=True, stop=True)
                nc.tensor.matmul(
                    lt[:, 1, hsl], lhsT=g_sb[:, j0:j0 + P],
                    rhs=f2_sb[:, hsl], start=True, stop=True)
                nc.scalar.activation(
                    out=et[:, :, hsl], in_=lt[:, :, hsl],
                    func=AF.Exp, bias=shiftb[:, 0:1], scale=1.0)

            def emit_transposes(ch):
                i0 = ch * CHUNK
                for jt in range(i0 // P, (i0 + CHUNK) // P):
                    pT = scratch.tile([P, IBLK], BF16, tag="sc", name="pT")
                    nc.tensor.transpose(
                        pT[:, 0:D], f1h_sb[D:P, jt * P:(jt + 1) * P],
                        ident2[D:P, 0:D])
                    nc.vector.tensor_copy(
                        out=hT_sb[:, jt, 0:D], in_=pT[:, 0:D])

            emit_proj(0)

            # ---------------- output projection ----------------
            # one channel-tile per call so the work spreads across j-loop
            # iterations instead of stalling PE in one burst
            def emit_out_start(ea, i0, w):
                return {"ea": ea, "i0": i0, "w": w}

            def emit_out_ct(st, ct, osb4=None):
                ea, i0, w = st["ea"], st["i0"], st["w"]
                osb = (osb4[:, ct, :] if osb4 is not None else
                       osbp.tile([P, IBLK], F32, tag="osb", name="osb"))
                # osb = z + cv first (waits only on the input stream),
                # then += the value GEMM result
                nc.vector.tensor_scalar_add(
                    out=osb[:, :w], in0=zin_sb[:, ct, i0:i0 + w],
                    scalar1=smalls_sb[:, 4 + ct:5 + ct])
                ops = scratch.tile([P, IBLK], F32, tag="sc", name="ops")
                nc.tensor.matmul(
                    ops[:, :w], lhsT=WvS_sb[:, ct, :],
                    rhs=ea[:, :w], start=True, stop=True)
                nc.vector.tensor_add(
                    osb[:, :w], osb[:, :w], ops[:, :w])
                if osb4 is None:
                    nc.sync.dma_start(
                        out=out_r[:, ct, i0:i0 + w], in_=osb[:, :w])

            # ---------------- attention main loop ----------------
            pending = None
            for ib, (i0, w) in enumerate(IBLOCKS):
                isl = slice(i0, i0 + w)
                num1 = nump.tile([D + 1, IBLK], F32, tag="num", name="num1")
                num2 = nump.tile([D + 1, IBLK], F32, tag="num", name="num2")
                prev_et = None
                # num consumes et with a 2-iteration lag so the logits for
                # jt+1 are never queued behind matmuls that wait on a fresh
                # exp result (PE stays an exp ahead of Act)
                et_hist = [None, None]
                jt_start = 0
                if ib == 0:
                    # prologue: first two j-tiles in 256-col halves, pipelined
                    # against the arrival of projection chunks 1 and 2 so the
                    # first exps fire as soon as chunk 0 is projected
                    lt0 = ltp.tile([P, 2, IBLK], F32, tag="lt", name="lt")
                    et0 = ebuf.tile([P, 2, IBLK], BF16, tag="et", name="et")
                    lt1 = ltp.tile([P, 2, IBLK], F32, tag="lt", name="lt")
                    et1 = ebuf.tile([P, 2, IBLK], BF16, tag="et", name="et")
                    emit_logits_half(lt0, et0, 0, 0, CHUNK)
                    emit_logits_half(lt1, et1, 1, 0, CHUNK)
                    emit_proj(1)
                    emit_transposes(0)
                    emit_transposes(1)
                    emit_logits_half(lt0, et0, 0, CHUNK, CHUNK)
                    emit_logits_half(lt1, et1, 1, CHUNK, CHUNK)
                    emit_proj(2)
                    emit_transposes(2)
                    et_hist = [et0, et1]
                    jt_start = 2
                for jt in range(jt_start, NJT + 2):
                    if ib == 0 and jt % 2 == 0 and 4 <= jt <= 14:
                        emit_proj(jt // 2 + 1)
                        emit_transposes(jt // 2 + 1)
                    if jt < NJT:
                        j0 = jt * P
                        lt = ltp.tile([P, 2, IBLK], F32, tag="lt", name="lt")
                        et = ebuf.tile([P, 2, IBLK], BF16, tag="et", name="et")
                        nc.tensor.matmul(
                            lt[:, 0, :w], lhsT=g_sb[:, j0:j0 + P],
                            rhs=f1h_sb[0:D, isl], start=True, stop=True)
                        nc.tensor.matmul(
                            lt[:, 1, :w], lhsT=g_sb[:, j0:j0 + P],
                            rhs=f2_sb[:, isl], start=True, stop=True)
                        nc.scalar.activation(
                            out=et[:, :, :w], in_=lt[:, :, :w],
                            func=AF.Exp, bias=shiftb[:, 0:1], scale=1.0)
                    if jt >= 2:
                        pj = jt - 2
                        st, sp = (pj == 0), (pj == NJT - 1)
                        nc.tensor.matmul(
                            num1[:, :w], lhsT=hT_sb[:, pj, :],
                            rhs=et_hist[0][:, 0, :w], start=st, stop=sp)
                        nc.tensor.matmul(
                            num2[:, :w], lhsT=hT_sb[:, pj, :],
                            rhs=et_hist[0][:, 1, :w], start=st, stop=sp)
                    if jt < NJT:
                        et_hist = [et_hist[1], et]
                    else:
                        et_hist = [et_hist[1], None]
                    if pending is not None and jt in (6, 8, 10, 12):
                        emit_out_ct(pending, (jt - 6) // 2)
                        if jt == 12:
                            pending = None
                rcp1 = rcpp.tile([1, IBLK], F32R, tag="rcp", name="rcp1")
                rcp2 = rcpp.tile([1, IBLK], F32R, tag="rcp", name="rcp2")
                with nc.allow_low_precision(
                        reason="softmax denominator reciprocal in f32r"):
                    nc.vector.reciprocal(rcp1[0:1, :w], num1[D:D + 1, :w])
                    nc.vector.reciprocal(rcp2[0:1, :w], num2[D:D + 1, :w])
                rb1 = scratch.tile([P, IBLK], F32, tag="sc", name="rb1")
                nc.tensor.matmul(
                    rb1[0:D, :w], lhsT=sel2[0:1, 0:D], rhs=rcp1[:, :w],
                    start=True, stop=True)
                rb2 = scratch.tile([P, IBLK], F32, tag="sc", name="rb2")
                nc.tensor.matmul(
                    rb2[0:D, :w], lhsT=sel2[0:1, 0:D], rhs=rcp2[:, :w],
                    start=True, stop=True)
                ea = eap.tile([P, IBLK], F32R, tag="ea", name="ea")
                if ib == len(IBLOCKS) - 1:
                    # last block: Act is idle after the final exp — run the
                    # copies there so DVE only serializes rcp+mul in the tail
                    nc.scalar.activation(
                        out=ea[0:D, :w], in_=num1[0:D, :w], func=AF.Copy)
                    nc.scalar.activation(
                        out=ea[D:P, :w], in_=num2[0:D, :w], func=AF.Copy)
                else:
                    nc.vector.tensor_copy(out=ea[0:D, :w], in_=num1[0:D, :w])
                    nc.vector.tensor_copy(out=ea[D:P, :w], in_=num2[0:D, :w])
                nc.vector.tensor_mul(ea[0:D, :w], ea[0:D, :w], rb1[0:D, :w])
                nc.vector.tensor_mul(ea[D:P, :w], ea[D:P, :w], rb2[0:D, :w])
                pending = emit_out_start(ea, i0, w)
            # final flush: one combined DMA (4 separate out-DMAs would
            # serialize ~0.7us apiece of HWDGE+transfer in the tail)
            wlast = pending["w"]
            osb4 = osbp.tile([P, NCT, IBLK], F32, tag="osb4", name="osb4",
                             bufs=1)
            for ct in range(NCT):
                emit_out_ct(pending, ct, osb4=osb4)
            i0 = pending["i0"]
            nc.sync.dma_start(
                out=out_r[:, :, i0:i0 + wlast], in_=osb4[:, :, :wlast])

    nc.compile()
    return nc


_NC_CACHE = None


def _get_nc():
    global _NC_CACHE
    if _NC_CACHE is None:
        _NC_CACHE = build_program()
    return _NC_CACHE


def _run(inputs, trace=False, trace_cores=None):
    from concourse.bass_utils import run_bass_kernel_spmd

    import ml_dtypes
    g = {k: np.ascontiguousarray(np.asarray(v, dtype=np.float32))
         for k, v in inputs.items()}
    x = g["x"].reshape(BS, C, N)
    y = g["y"].reshape(BS, C, N)
    x16 = np.ascontiguousarray(x.astype(ml_dtypes.bfloat16))
    y16 = np.ascontiguousarray(y.astype(ml_dtypes.bfloat16))

    def core_inputs(b, s):
        def sel(a0, a1):
            return a0 if s == 0 else a1

        gate1 = float(np.asarray(sel(g["alpha"], g["gamma"])).reshape(-1)[0])
        gate2 = float(np.asarray(sel(g["beta"], g["sigma"])).reshape(-1)[0])
        Wcat = np.concatenate(
            [sel(g["Wg1"], g["Wg2"]).T, g["Wf1"].T,
             sel(g["Wh1"], g["Wh2"]).T, g["Wf2"].T], axis=1)   # [C, 256]
        WvS = np.concatenate(
            [gate1 * sel(g["Wv11"], g["Wv12"]).T,
             gate2 * sel(g["Wv21"], g["Wv22"]).T], axis=0)     # [128, C]
        cv = (gate1 * sel(g["bv11"], g["bv12"])
              + gate2 * sel(g["bv21"], g["bv22"]))             # [C]
        smalls = np.zeros((P, 8), np.float32)
        smalls[0:D, 0] = g["bf1"]
        smalls[D:P, 0] = sel(g["bh1"], g["bh2"])
        smalls[0:D, 1] = g["bf2"]
        smalls[0:D, 2] = sel(g["bg1"], g["bg2"])
        smalls[:, 4:8] = cv.reshape(NCT, P).T
        sel2 = np.zeros((2, P), np.float32)
        sel2[0, 0:D] = 1.0
        sel2[1, D:P] = 1.0
        return {
            "xin": np.ascontiguousarray(x16[b]),
            "yin": np.ascontiguousarray(y16[b]),
            "zin": np.ascontiguousarray(sel(x16, y16)[b]),
            "Wcat": np.ascontiguousarray(Wcat.astype(ml_dtypes.bfloat16)),
            "WvS": np.ascontiguousarray(WvS),
            "smalls": smalls,
            "sel": sel2,
        }

    in_maps = [core_inputs(core // 2, core % 2) for core in range(8)]
    res = run_bass_kernel_spmd(
        _get_nc(), in_maps, core_ids=list(range(8)), trace=trace,
        trace_cores=trace_cores)
    outs = [r["out"] for r in res.results]
    x_out = np.stack([outs[2 * b] for b in range(BS)]).reshape(BS, C, H, W)
    y_out = np.stack([outs[2 * b + 1] for b in range(BS)]).reshape(BS, C, H, W)
    return (x_out, y_out), res


def kernel(**inputs):
    out, _ = _run(inputs)
    return out
